# revision 28
# baseline (speedup 1.0000x reference)
"""Bass/Trainium2 kernel for nn_BipartiteLayer (gnn_message_passing), v2.

Math (see reference):
  xp    = x @ W_in.T                             [N, F]    F=128 (b_in=0)
  score = exp(-|xp @ W_a.T|)                     [N, A]    A=8   (b_a=0)
  e     = score[:, :, None] * xp[:, None, :]     [N, A, F]
  mean_p/max_p = segment mean / max of e by batch -> [B, A, F]
  out   = relu([x, xp, agg[batch]] @ W_out.T)    [N, 64]   (b_out=0)

Restructured as in v1:
  out = relu(x @ Wx.T + xp @ Wxp.T + proj[batch]) where
  proj[b] = sum_a mean[b,a] @ Wm_a.T + max[b,a] @ Wxx_a.T   (tiny [B,64])
  and the per-node gather of proj is a one-hot matmul G @ proj on PE.

v2 layout: 512 segments dealt by descending count to 8 cores x 64 slots,
every slot padded to a UNIFORM width W=256 (= 2 partition tiles), so
- each 128-col node tile belongs to exactly one slot (clean segment-sum
  matmuls, no sub-blocks),
- the segment max runs as a halving TT-max tree over 3D APs
  [128, slots, W] with only ~9 DVE instructions per (a, super-chunk).
Everything flows in bf16 (tolerance is 2e-2; bf16 keeps DVE in 2x mode and
halves DMA); PSUM accumulation stays f32.

Score broadcast to 128 partitions (for e = xpT * score_a) is routed per a:
  'pe'  : one-hot matmul -> PSUM, ACT copies to SBUF bf16
  'gp'  : gpsimd partition_broadcast straight to SBUF bf16
  'dma' : SBUF->SBUF DMA with a stride-0 partition source
"""

import sys

sys.path.insert(0, "/opt/trn_rl_repo")

import numpy as np
import ml_dtypes

BF16 = np.dtype(ml_dtypes.bfloat16)

N_GLOBAL, D_IN, D_OUT, A, B = 100000, 128, 64, 8, 512
F = 2 * D_OUT  # 128
NCORES = 8
J = B // NCORES          # 64 slots per core
W = 256                  # uniform padded slot width (2 tiles)
NPAD = J * W             # 16384 padded node columns per core
CH = 512                 # chunk: PSUM-sized column block = 2 slots, 4 tiles
SC = 2048                # super-chunk: 4 chunks = 8 slots = 16 tiles
NCH = NPAD // CH         # 32
NSC = NPAD // SC         # 8
CPS = SC // CH           # chunks per super = 4
SPS = SC // W            # slots per super = 8
TPS = SC // 128          # tiles per super = 16

# score-broadcast route per a-channel
ROUTES = ("pe",) * 8

_cache = {}


# ---- custom DVE op: out = in0*in1, accum_out = per-partition max(out) ----
# (stock InstTensorTensorReduce crashes the exec unit on this stack, so the
# fused multiply+max-reduce is registered as a runtime custom-DVE op)
_MULMAX_NAME = "TT_MULMAX_ANT"


def _mulmax_ref(in0, in1, s0, s1, imm2):
    b = (in0.astype(np.float32) * in1).astype(np.float32)
    mx = np.nanmax(
        np.where(np.isnan(b), -np.finfo(np.float32).max, b).reshape(b.shape[0], -1),
        axis=-1, keepdims=True)
    return b, mx


def _get_mulmax():
    import dataclasses
    from concourse import dve_ops
    from concourse.dve_spec import Spec, Src0, Src1, lower, _has_src1, maxx
    from concourse.dve_uop import DveOpSpec

    for op in dve_ops.OPS:
        if op.name == _MULMAX_NAME:
            return op
    spec = Spec(body=Src0 * Src1, accum=maxx, reference=_mulmax_ref)
    tmp = dve_ops.DveOp(_MULMAX_NAME, spec, subdim=False, uops_sha={})
    dve_ops.OPS.append(tmp)
    dve_ops._SUB_OPCODE_FOR_NAME[_MULMAX_NAME] = (
        dve_ops._CUSTOM_DVE_ROW_BASE + len(dve_ops.OPS) - 1)
    dve_ops.CUSTOM_DVE_SPECS[_MULMAX_NAME] = spec
    opcode = dve_ops.get_dve_sub_opcode(_MULMAX_NAME)
    shas = {}
    for ver in ("v3", "v4"):
        s = DveOpSpec(name=_MULMAX_NAME, opcode=opcode, uops=lower(spec, ver=ver),
                      rd1_en=_has_src1(spec))
        shas[ver] = s.sha(ver)
    final = dataclasses.replace(tmp, uops_sha=shas)
    dve_ops.OPS[-1] = final
    return final


def _build_program(chunks=None, n_pad=None, dma="sync", reps=1, ablate=(),
                   routes=ROUTES, maxmode="mulmax", fd=None):
    import concourse.bacc as bacc
    import concourse.tile as tile
    from concourse import mybir
    from concourse.masks import make_identity

    MULMAX = _get_mulmax()
    if fd is None:
        fd = (W,) * J

    f32 = mybir.dt.float32
    bf16 = mybir.dt.bfloat16
    AF = mybir.ActivationFunctionType
    OP = mybir.AluOpType

    nc = bacc.Bacc("TRN2", target_bir_lowering=False, debug=False,
                   num_devices=NCORES)
    dma_eng = getattr(nc, dma)

    xT_d = nc.dram_tensor("xT", [128, NPAD], bf16, kind="ExternalInput")
    G_d = nc.dram_tensor("G", [J, NPAD], bf16, kind="ExternalInput")
    invc_d = nc.dram_tensor("invc", [J, 1], f32, kind="ExternalInput")
    WinT_d = nc.dram_tensor("WinT", [128, 128], bf16, kind="ExternalInput")
    WaT_d = nc.dram_tensor("WaT", [128, A], bf16, kind="ExternalInput")
    WxT_d = nc.dram_tensor("WxT", [128, 64], bf16, kind="ExternalInput")
    WxpT_d = nc.dram_tensor("WxpT", [128, 64], bf16, kind="ExternalInput")
    WmT_d = nc.dram_tensor("WmT", [128, A * 64], f32, kind="ExternalInput")
    WxxT_d = nc.dram_tensor("WxxT", [128, A * 64], f32, kind="ExternalInput")
    sel_d = nc.dram_tensor("sel", [A, A * 128], bf16, kind="ExternalInput")
    y_d = nc.dram_tensor("y", [64, NPAD], bf16, kind="ExternalOutput")

    from contextlib import ExitStack

    with tile.TileContext(nc) as tc, ExitStack() as ctx:
        consts = ctx.enter_context(tc.tile_pool(name="consts", bufs=1))
        big = ctx.enter_context(tc.tile_pool(name="big", bufs=1))

        use_sum = "sum" not in ablate
        use_max = "max" not in ablate
        use_D = "phaseD" not in ablate
        ident = None
        if use_sum:
            ident = consts.tile([128, 128], bf16)
            make_identity(nc, ident)
        WinT = consts.tile([128, 128], bf16)
        dma_eng.dma_start(out=WinT, in_=WinT_d[:])
        WaT = consts.tile([128, A], bf16)
        dma_eng.dma_start(out=WaT, in_=WaT_d[:])
        WxT = consts.tile([128, 64], bf16)
        dma_eng.dma_start(out=WxT, in_=WxT_d[:])
        WxpT = consts.tile([128, 64], bf16)
        dma_eng.dma_start(out=WxpT, in_=WxpT_d[:])
        WmT = consts.tile([128, A, 64], f32)
        dma_eng.dma_start(out=WmT, in_=WmT_d[:].rearrange("p (a o) -> p a o", a=A))
        WxxT = consts.tile([128, A, 64], f32)
        dma_eng.dma_start(out=WxxT, in_=WxxT_d[:].rearrange("p (a o) -> p a o", a=A))
        invc = consts.tile([J, 1], f32)
        dma_eng.dma_start(out=invc, in_=invc_d[:])
        if use_max:
            sel = consts.tile([A, A, 128], bf16)
            dma_eng.dma_start(out=sel,
                              in_=sel_d[:].rearrange("k (a m) -> k a m", a=A))

        xT = big.tile([128, NPAD], bf16)
        dma_eng.dma_start(out=xT, in_=xT_d[:])

        repbig = ctx.enter_context(tc.tile_pool(name="repbig", bufs=2))
        rep1 = ctx.enter_context(tc.tile_pool(name="rep1", bufs=1))
        scbp = ctx.enter_context(tc.tile_pool(name="scb", bufs=2))
        epool = ctx.enter_context(tc.tile_pool(name="epool", bufs=2))
        nm = ctx.enter_context(tc.tile_pool(name="nm", bufs=2))
        sa = ctx.enter_context(tc.tile_pool(name="sa", bufs=2))

        for _rep in range(reps):
          xpT = repbig.tile([128, NPAD], bf16, tag="xpT")
          segsum = repbig.tile([128, A, J], f32, tag="segsum")  # [f, a, slot]
          segmax = repbig.tile([128, A, J], f32, tag="segmax")
          with (
              tc.tile_pool(name="psA", bufs=2, space="PSUM") as psA,
              tc.tile_pool(name="psP", bufs=1, space="PSUM") as psP,
              tc.tile_pool(name="psT", bufs=1, space="PSUM") as psT,
              tc.tile_pool(name="psS", bufs=1, space="PSUM") as psS,
              tc.tile_pool(name="psB", bufs=3, space="PSUM") as psB,
          ):
            for s in range(NSC):
              s0 = s * SC
              scoreT = sa.tile([A, SC], bf16, tag="scoreT")
              if "sum" not in ablate:
                  xp_nm = nm.tile([128, TPS, 128], bf16, tag="xp_nm")
                  sc_nm = nm.tile([128, TPS, A], bf16, tag="sc_nm")
              if "max" not in ablate and maxmode == "mulmax":
                  escr = epool.tile([128, W], bf16, tag="escr")
              elif "max" not in ablate:
                  scb = scbp.tile([128, A, SC], bf16, tag="scb")
              for c in range(CPS):
                c0 = s0 + c * CH
                xp_ps = psA.tile([128, CH], f32, tag="xp")
                nc.tensor.matmul(xp_ps[:], lhsT=WinT[:], rhs=xT[:, c0:c0 + CH],
                                 start=True, stop=True)
                nc.scalar.copy(xpT[:, c0:c0 + CH], xp_ps[:])

                pre_ps = psP.tile([A, CH], f32, tag="pre")
                nc.tensor.matmul(pre_ps[:], lhsT=WaT[:], rhs=xpT[:, c0:c0 + CH],
                                 start=True, stop=True)
                sabs = sa.tile([A, CH], f32, tag="sabs")
                nc.scalar.activation(sabs[:], pre_ps[:], AF.Abs)
                nc.scalar.activation(scoreT[:, c * CH:(c + 1) * CH], sabs[:],
                                     AF.Exp, scale=-1.0)

                if "sum" not in ablate:
                    tr_ps = psT.tile([128, CH + 4 * A], bf16, tag="tr")
                    for t in range(4):
                        nc.tensor.transpose(
                            tr_ps[:, t * 128:(t + 1) * 128],
                            xpT[:, c0 + t * 128:c0 + (t + 1) * 128], ident[:])
                        nc.tensor.transpose(
                            tr_ps[:, CH + t * A:CH + (t + 1) * A],
                            scoreT[:, c * CH + t * 128:c * CH + (t + 1) * 128],
                            ident[:A, :A])
                    tbase = c * 4
                    nc.scalar.copy(
                        xp_nm[:, tbase:tbase + 4, :].rearrange(
                            "p t f -> p (t f)"),
                        tr_ps[:, :CH])
                    nc.scalar.copy(
                        sc_nm[:, tbase:tbase + 4, :],
                        tr_ps[:, CH:].rearrange("p (t a) -> p t a", t=4))
                    ss_ps = psS.tile([128, 2, A], f32, tag="ss")
                    for sl in range(2):
                        for t2 in range(2):
                            t = tbase + sl * 2 + t2
                            nc.tensor.matmul(
                                ss_ps[:, sl, :], lhsT=xp_nm[:, t, :],
                                rhs=sc_nm[:, t, :],
                                start=(t2 == 0), stop=(t2 == 1))
                    slot0 = c0 // W
                    nc.scalar.copy(segsum[:, :, slot0:slot0 + 2]
                                   .rearrange("p a s -> p s a"), ss_ps[:])

              if "max" not in ablate and maxmode == "mulmax":
                # fused multiply+max straight from PSUM; no movers, no e.
                for a in range(A):
                    for c in range(CPS):
                        c0 = s0 + c * CH
                        g2 = c0 // W
                        bw = W + fd[g2 + 1]
                        sb_ps = psB.tile([128, CH], f32, tag="scB")
                        nc.tensor.matmul(sb_ps[:, :bw], lhsT=sel[:, a, :],
                                         rhs=scoreT[:, c * CH:c * CH + bw],
                                         start=True, stop=True)
                        for sl in range(2):
                            slot = g2 + sl
                            fdj = fd[slot]
                            nc.vector._custom_dve(
                                MULMAX,
                                out=escr[:, :fdj],
                                in0=xpT[:, c0 + sl * W:c0 + sl * W + fdj],
                                in1=sb_ps[:, sl * W:sl * W + fdj],
                                accum_out=segmax[:, a, slot:slot + 1])
              elif "max" not in ablate:
                for a in range(A):
                    r = routes[a]
                    if r == "pe":
                        for c in range(CPS):
                            c0 = s0 + c * CH
                            sb_ps = psB.tile([128, CH], f32, tag="scB")
                            nc.tensor.matmul(sb_ps[:], lhsT=sel[:, a, :],
                                             rhs=scoreT[:, c * CH:(c + 1) * CH],
                                             start=True, stop=True)
                            nc.scalar.copy(
                                scb[:, a, c * CH:(c + 1) * CH], sb_ps[:])
                    elif r == "gp":
                        nc.gpsimd.partition_broadcast(
                            scb[:, a, :], scoreT[a:a + 1, :])
                    else:  # dma
                        nc.sync.dma_start(
                            out=scb[:, a, :],
                            in_=scoreT[a:a + 1, :]
                            .rearrange("p (x n) -> p x n", x=1)
                            .broadcast_to((1, 128, SC)))

                for a in range(A):
                    e = epool.tile([128, SPS, W], bf16, tag="e")
                    if "prod" not in ablate:
                        nc.vector.tensor_tensor(
                            out=e[:].rearrange("p k w -> p (k w)"),
                            in0=xpT[:, s0:s0 + SC],
                            in1=scb[:, a, :], op=OP.mult)
                    w = W
                    while w > 2:
                        w2 = w // 2
                        nc.vector.tensor_tensor(
                            out=e[:, :, :w2], in0=e[:, :, :w2],
                            in1=e[:, :, w2:w], op=OP.max)
                        w = w2
                    nc.vector.tensor_tensor(
                        out=segmax[:, a, s * SPS:(s + 1) * SPS],
                        in0=e[:, :, 0], in1=e[:, :, 1], op=OP.max)

          # --- per-segment aggregates -> proj [J, 64] ---
          with (
              tc.tile_pool(name="psC", bufs=2, space="PSUM") as psC,
              tc.tile_pool(name="small", bufs=2) as small,
          ):
              pm = psC.tile([J, 64], f32, tag="proj")
              for a in range(A):
                  nc.tensor.matmul(pm[:], lhsT=segsum[:, a, :], rhs=WmT[:, a, :],
                                   start=(a == 0), stop=(a == A - 1))
              px = psC.tile([J, 64], f32, tag="proj")
              for a in range(A):
                  nc.tensor.matmul(px[:], lhsT=segmax[:, a, :], rhs=WxxT[:, a, :],
                                   start=(a == 0), stop=(a == A - 1))
              proj = small.tile([J, 64], f32, tag="proj_sb")
              nc.vector.tensor_scalar(out=proj[:], in0=pm[:], scalar1=invc[:],
                                      scalar2=None, op0=OP.mult)
              nc.vector.tensor_tensor(out=proj[:], in0=proj[:], in1=px[:],
                                      op=OP.add)
              proj_r = small.tile([J, 64], bf16, tag="proj_r")
              nc.scalar.copy(proj_r[:], proj[:])

              # --- final projection + gather + relu ---
              with (
                  tc.tile_pool(name="psD", bufs=2, space="PSUM") as psD,
                  tc.tile_pool(name="gp", bufs=3) as gpp,
                  tc.tile_pool(name="yp", bufs=2) as yp,
              ):
                  if "phaseD" not in ablate:
                    for g in range(NCH):
                      c0 = g * CH
                      bw = W + fd[2 * g + 1]
                      gt = gpp.tile([J, CH], bf16, tag="gt")
                      dma_eng.dma_start(out=gt[:, :bw], in_=G_d[:, c0:c0 + bw])
                      yT_ps = psD.tile([64, CH], f32, tag="yT")
                      nc.tensor.matmul(yT_ps[:, :bw], lhsT=WxT[:],
                                       rhs=xT[:, c0:c0 + bw],
                                       start=True, stop=False)
                      nc.tensor.matmul(yT_ps[:, :bw], lhsT=WxpT[:],
                                       rhs=xpT[:, c0:c0 + bw],
                                       start=False, stop=False)
                      nc.tensor.matmul(yT_ps[:, :bw], lhsT=proj_r[:],
                                       rhs=gt[:, :bw],
                                       start=False, stop=True)
                      yT_sb = yp.tile([64, CH], bf16, tag="yT_sb")
                      nc.scalar.activation(yT_sb[:, :bw], yT_ps[:, :bw], AF.Relu)
                      nc.sync.dma_start(out=y_d[:, c0:c0 + bw], in_=yT_sb[:, :bw])
    nc.compile()
    return nc


def _prep(x, batch, W_in, b_in, W_a, b_a, W_out, b_out):
    x = np.asarray(x, np.float32)
    batch = np.asarray(batch).astype(np.int64)
    counts = np.bincount(batch, minlength=B).astype(np.int64)
    assert counts.max() <= W, f"segment count {counts.max()} > W={W}"
    seg_start = np.zeros(B + 1, np.int64)
    np.cumsum(counts, out=seg_start[1:])

    order = np.argsort(-counts, kind="stable")
    fd = np.zeros(J, np.int64)
    for j in range(J):
        mx = int(counts[order[j * NCORES:(j + 1) * NCORES]].max())
        fd[j] = min(W, max(32, -(-mx // 32) * 32))
    _prep.last_fd = tuple(int(v) for v in fd)

    W_out = np.asarray(W_out, np.float32)
    WmT = np.empty((128, A, 64), np.float32)
    WxxT = np.empty((128, A, 64), np.float32)
    for a in range(A):
        base = D_IN + F + a * 2 * F
        WmT[:, a, :] = W_out[:, base:base + F].T
        WxxT[:, a, :] = W_out[:, base + F:base + 2 * F].T

    for nm, v in (("b_in", b_in), ("b_a", b_a), ("b_out", b_out)):
        assert np.abs(np.asarray(v, np.float32)).max() == 0.0, f"{nm} != 0"

    shared = {
        "WinT": np.ascontiguousarray(np.asarray(W_in, np.float32).T).astype(BF16),
        "WaT": np.ascontiguousarray(np.asarray(W_a, np.float32).T).astype(BF16),
        "WxT": np.ascontiguousarray(W_out[:, :D_IN].T).astype(BF16),
        "WxpT": np.ascontiguousarray(W_out[:, D_IN:D_IN + F].T).astype(BF16),
        "WmT": np.ascontiguousarray(WmT.reshape(128, A * 64)),
        "WxxT": np.ascontiguousarray(WxxT.reshape(128, A * 64)),
        "sel": np.ascontiguousarray(
            np.repeat(np.eye(A, dtype=np.float32), 128, axis=1)).astype(BF16),
    }

    in_maps, gathers = [], []
    for c in range(NCORES):
        xT_c = np.zeros((128, NPAD), np.float32)
        G_c = np.zeros((J, NPAD), np.float32)
        invc_c = np.zeros((J, 1), np.float32)
        src_all, dst_all = [], []
        for j in range(J):
            seg = int(order[j * NCORES + c])
            n = int(counts[seg])
            invc_c[j] = 1.0 / max(n, 1)
            if n == 0:
                continue
            s0 = int(seg_start[seg])
            o = j * W
            src_all.append(np.arange(s0, s0 + n))
            dst_all.append(np.arange(o, o + n))
            G_c[j, o:o + n] = 1.0
        src = np.concatenate(src_all)
        dst = np.concatenate(dst_all)
        xT_c[:, dst] = x[src].T
        in_maps.append({"xT": xT_c.astype(BF16), "G": G_c.astype(BF16),
                        "invc": invc_c, **shared})
        gathers.append((src, dst))
    return None, NPAD, in_maps, gathers


def kernel(x, batch, num_segments, W_in, b_in, W_a, b_a, W_out, b_out,
           _trace=False):
    from concourse.bass_utils import run_bass_kernel_spmd

    assert int(num_segments) == B
    _, _, in_maps, gathers = _prep(
        x, batch, W_in, b_in, W_a, b_a, W_out, b_out)

    key = ("prog", _prep.last_fd)
    if key not in _cache:
        _cache[key] = _build_program(fd=_prep.last_fd)
    nc = _cache[key]

    res = run_bass_kernel_spmd(nc, in_maps, core_ids=list(range(NCORES)),
                               trace=_trace)
    out = np.empty((N_GLOBAL, D_OUT), np.float32)
    for c in range(NCORES):
        src, dst = gathers[c]
        y = np.asarray(res.results[c]["y"])  # [64, NPAD] bf16
        out[src] = y[:, dst].T.astype(np.float32)
    kernel._last_result = res
    return out


# revision 29
# speedup vs baseline: 1.1082x; 1.1082x over previous
"""Bass/Trainium2 kernel for nn_BipartiteLayer (gnn_message_passing), v2.

Math (see reference):
  xp    = x @ W_in.T                             [N, F]    F=128 (b_in=0)
  score = exp(-|xp @ W_a.T|)                     [N, A]    A=8   (b_a=0)
  e     = score[:, :, None] * xp[:, None, :]     [N, A, F]
  mean_p/max_p = segment mean / max of e by batch -> [B, A, F]
  out   = relu([x, xp, agg[batch]] @ W_out.T)    [N, 64]   (b_out=0)

Restructured as in v1:
  out = relu(x @ Wx.T + xp @ Wxp.T + proj[batch]) where
  proj[b] = sum_a mean[b,a] @ Wm_a.T + max[b,a] @ Wxx_a.T   (tiny [B,64])
  and the per-node gather of proj is a one-hot matmul G @ proj on PE.

v2 layout: 512 segments dealt by descending count to 8 cores x 64 slots,
every slot padded to a UNIFORM width W=256 (= 2 partition tiles), so
- each 128-col node tile belongs to exactly one slot (clean segment-sum
  matmuls, no sub-blocks),
- the segment max runs as a halving TT-max tree over 3D APs
  [128, slots, W] with only ~9 DVE instructions per (a, super-chunk).
Everything flows in bf16 (tolerance is 2e-2; bf16 keeps DVE in 2x mode and
halves DMA); PSUM accumulation stays f32.

Score broadcast to 128 partitions (for e = xpT * score_a) is routed per a:
  'pe'  : one-hot matmul -> PSUM, ACT copies to SBUF bf16
  'gp'  : gpsimd partition_broadcast straight to SBUF bf16
  'dma' : SBUF->SBUF DMA with a stride-0 partition source
"""

import sys

sys.path.insert(0, "/opt/trn_rl_repo")

import numpy as np
import ml_dtypes

BF16 = np.dtype(ml_dtypes.bfloat16)

N_GLOBAL, D_IN, D_OUT, A, B = 100000, 128, 64, 8, 512
F = 2 * D_OUT  # 128
NCORES = 8
J = B // NCORES          # 64 slots per core
W = 256                  # uniform padded slot width (2 tiles)
NPAD = J * W             # 16384 padded node columns per core
CH = 512                 # chunk: PSUM-sized column block = 2 slots, 4 tiles
SC = 2048                # super-chunk: 4 chunks = 8 slots = 16 tiles
NCH = NPAD // CH         # 32
NSC = NPAD // SC         # 8
CPS = SC // CH           # chunks per super = 4
SPS = SC // W            # slots per super = 8
TPS = SC // 128          # tiles per super = 16

# score-broadcast route per a-channel
ROUTES = ("pe",) * 8

_cache = {}


# ---- custom DVE op: out = in0*in1, accum_out = per-partition max(out) ----
# (stock InstTensorTensorReduce crashes the exec unit on this stack, so the
# fused multiply+max-reduce is registered as a runtime custom-DVE op)
_MULMAX_NAME = "TT_MULMAX_ANT"


def _mulmax_ref(in0, in1, s0, s1, imm2):
    b = (in0.astype(np.float32) * in1).astype(np.float32)
    mx = np.nanmax(
        np.where(np.isnan(b), -np.finfo(np.float32).max, b).reshape(b.shape[0], -1),
        axis=-1, keepdims=True)
    return b, mx


def _get_mulmax():
    import dataclasses
    from concourse import dve_ops
    from concourse.dve_spec import Spec, Src0, Src1, lower, _has_src1, maxx
    from concourse.dve_uop import DveOpSpec

    for op in dve_ops.OPS:
        if op.name == _MULMAX_NAME:
            return op
    spec = Spec(body=Src0 * Src1, accum=maxx, reference=_mulmax_ref)
    tmp = dve_ops.DveOp(_MULMAX_NAME, spec, subdim=False, uops_sha={})
    dve_ops.OPS.append(tmp)
    dve_ops._SUB_OPCODE_FOR_NAME[_MULMAX_NAME] = (
        dve_ops._CUSTOM_DVE_ROW_BASE + len(dve_ops.OPS) - 1)
    dve_ops.CUSTOM_DVE_SPECS[_MULMAX_NAME] = spec
    opcode = dve_ops.get_dve_sub_opcode(_MULMAX_NAME)
    shas = {}
    for ver in ("v3", "v4"):
        s = DveOpSpec(name=_MULMAX_NAME, opcode=opcode, uops=lower(spec, ver=ver),
                      rd1_en=_has_src1(spec))
        shas[ver] = s.sha(ver)
    final = dataclasses.replace(tmp, uops_sha=shas)
    dve_ops.OPS[-1] = final
    return final


def _build_program(chunks=None, n_pad=None, dma="sync", reps=1, ablate=(),
                   routes=ROUTES, maxmode="mulmax", fd=None):
    import concourse.bacc as bacc
    import concourse.tile as tile
    from concourse import mybir
    from concourse.masks import make_identity

    MULMAX = _get_mulmax()
    if fd is None:
        fd = (W,) * J

    f32 = mybir.dt.float32
    bf16 = mybir.dt.bfloat16
    AF = mybir.ActivationFunctionType
    OP = mybir.AluOpType

    nc = bacc.Bacc("TRN2", target_bir_lowering=False, debug=False,
                   num_devices=NCORES)
    dma_eng = getattr(nc, dma)

    xT_d = nc.dram_tensor("xT", [128, NPAD], bf16, kind="ExternalInput")
    G_d = nc.dram_tensor("G", [J, NPAD], bf16, kind="ExternalInput")
    invc_d = nc.dram_tensor("invc", [J, 1], f32, kind="ExternalInput")
    WinT_d = nc.dram_tensor("WinT", [128, 128], bf16, kind="ExternalInput")
    WaT_d = nc.dram_tensor("WaT", [128, A], bf16, kind="ExternalInput")
    WxT_d = nc.dram_tensor("WxT", [128, 64], bf16, kind="ExternalInput")
    WxpT_d = nc.dram_tensor("WxpT", [128, 64], bf16, kind="ExternalInput")
    WmT_d = nc.dram_tensor("WmT", [128, A * 64], f32, kind="ExternalInput")
    WxxT_d = nc.dram_tensor("WxxT", [128, A * 64], f32, kind="ExternalInput")
    sel_d = nc.dram_tensor("sel", [A, A * 128], bf16, kind="ExternalInput")
    y_d = nc.dram_tensor("y", [64, NPAD], bf16, kind="ExternalOutput")

    from contextlib import ExitStack

    with tile.TileContext(nc) as tc, ExitStack() as ctx:
        consts = ctx.enter_context(tc.tile_pool(name="consts", bufs=1))
        big = ctx.enter_context(tc.tile_pool(name="big", bufs=1))

        use_sum = "sum" not in ablate
        use_max = "max" not in ablate
        use_D = "phaseD" not in ablate
        ident = None
        if use_sum:
            ident = consts.tile([128, 128], bf16)
            make_identity(nc, ident)
        WinT = consts.tile([128, 128], bf16)
        dma_eng.dma_start(out=WinT, in_=WinT_d[:])
        WaT = consts.tile([128, A], bf16)
        dma_eng.dma_start(out=WaT, in_=WaT_d[:])
        WxT = consts.tile([128, 64], bf16)
        dma_eng.dma_start(out=WxT, in_=WxT_d[:])
        WxpT = consts.tile([128, 64], bf16)
        dma_eng.dma_start(out=WxpT, in_=WxpT_d[:])
        WmT = consts.tile([128, A, 64], f32)
        dma_eng.dma_start(out=WmT, in_=WmT_d[:].rearrange("p (a o) -> p a o", a=A))
        WxxT = consts.tile([128, A, 64], f32)
        dma_eng.dma_start(out=WxxT, in_=WxxT_d[:].rearrange("p (a o) -> p a o", a=A))
        invc = consts.tile([J, 1], f32)
        dma_eng.dma_start(out=invc, in_=invc_d[:])
        if use_max:
            sel = consts.tile([A, A, 128], bf16)
            dma_eng.dma_start(out=sel,
                              in_=sel_d[:].rearrange("k (a m) -> k a m", a=A))

        xT = big.tile([128, NPAD], bf16)
        dma_eng.dma_start(out=xT, in_=xT_d[:])

        repbig = ctx.enter_context(tc.tile_pool(name="repbig", bufs=2))
        rep1 = ctx.enter_context(tc.tile_pool(name="rep1", bufs=1))
        scbp = ctx.enter_context(tc.tile_pool(name="scb", bufs=2))
        epool = ctx.enter_context(tc.tile_pool(name="epool", bufs=2))
        nm = ctx.enter_context(tc.tile_pool(name="nm", bufs=2))
        sa = ctx.enter_context(tc.tile_pool(name="sa", bufs=2))

        for _rep in range(reps):
          xpT = repbig.tile([128, NPAD], bf16, tag="xpT")
          segsum = repbig.tile([128, A, J], f32, tag="segsum")  # [f, a, slot]
          segmax = repbig.tile([128, A, J], f32, tag="segmax")
          with (
              tc.tile_pool(name="psA", bufs=2, space="PSUM") as psA,
              tc.tile_pool(name="psP", bufs=1, space="PSUM") as psP,
              tc.tile_pool(name="psT", bufs=1, space="PSUM") as psT,
              tc.tile_pool(name="psS", bufs=1, space="PSUM") as psS,
              tc.tile_pool(name="psB", bufs=3, space="PSUM") as psB,
          ):
            for s in range(NSC):
              s0 = s * SC
              scoreT = sa.tile([A, SC], bf16, tag="scoreT")
              if "sum" not in ablate:
                  xp_nm = nm.tile([128, TPS, 128], bf16, tag="xp_nm")
                  sc_nm = nm.tile([128, TPS, A], bf16, tag="sc_nm")
              if "max" not in ablate and maxmode == "mulmax":
                  escr = epool.tile([128, W], bf16, tag="escr")
              elif "max" not in ablate:
                  scb = scbp.tile([128, A, SC], bf16, tag="scb")
              for c in range(CPS):
                c0 = s0 + c * CH
                xp_ps = psA.tile([128, CH], f32, tag="xp")
                nc.tensor.matmul(xp_ps[:], lhsT=WinT[:], rhs=xT[:, c0:c0 + CH],
                                 start=True, stop=True)
                nc.scalar.copy(xpT[:, c0:c0 + CH], xp_ps[:])

                pre_ps = psP.tile([A, CH], f32, tag="pre")
                nc.tensor.matmul(pre_ps[:], lhsT=WaT[:], rhs=xpT[:, c0:c0 + CH],
                                 start=True, stop=True)
                sabs = sa.tile([A, CH], f32, tag="sabs")
                nc.scalar.activation(sabs[:], pre_ps[:], AF.Abs)
                nc.scalar.activation(scoreT[:, c * CH:(c + 1) * CH], sabs[:],
                                     AF.Exp, scale=-1.0)

                if "sum" not in ablate:
                    tr_ps = psT.tile([128, CH + 4 * A], bf16, tag="tr")
                    for t in range(4):
                        nc.tensor.transpose(
                            tr_ps[:, t * 128:(t + 1) * 128],
                            xpT[:, c0 + t * 128:c0 + (t + 1) * 128], ident[:])
                        nc.tensor.transpose(
                            tr_ps[:, CH + t * A:CH + (t + 1) * A],
                            scoreT[:, c * CH + t * 128:c * CH + (t + 1) * 128],
                            ident[:A, :A])
                    tbase = c * 4
                    nc.scalar.copy(
                        xp_nm[:, tbase:tbase + 4, :].rearrange(
                            "p t f -> p (t f)"),
                        tr_ps[:, :CH])
                    nc.scalar.copy(
                        sc_nm[:, tbase:tbase + 4, :],
                        tr_ps[:, CH:].rearrange("p (t a) -> p t a", t=4))
                    ss_ps = psS.tile([128, 2, A], f32, tag="ss")
                    for sl in range(2):
                        for t2 in range(2):
                            t = tbase + sl * 2 + t2
                            nc.tensor.matmul(
                                ss_ps[:, sl, :], lhsT=xp_nm[:, t, :],
                                rhs=sc_nm[:, t, :],
                                start=(t2 == 0), stop=(t2 == 1))
                    slot0 = c0 // W
                    nc.scalar.copy(segsum[:, :, slot0:slot0 + 2]
                                   .rearrange("p a s -> p s a"), ss_ps[:])

              if "max" not in ablate and maxmode == "mulmax":
                # fused multiply+max straight from PSUM; no movers, no e.
                for a in range(A):
                    for c in range(CPS):
                        c0 = s0 + c * CH
                        g2 = c0 // W
                        bw = W + fd[g2 + 1]
                        sb_ps = psB.tile([128, CH], f32, tag="scB")
                        nc.tensor.matmul(sb_ps[:, :bw], lhsT=sel[:, a, :],
                                         rhs=scoreT[:, c * CH:c * CH + bw],
                                         start=True, stop=True)
                        for sl in range(2):
                            slot = g2 + sl
                            fdj = fd[slot]
                            nc.vector._custom_dve(
                                MULMAX,
                                out=escr[:, :fdj],
                                in0=xpT[:, c0 + sl * W:c0 + sl * W + fdj],
                                in1=sb_ps[:, sl * W:sl * W + fdj],
                                accum_out=segmax[:, a, slot:slot + 1])
              elif "max" not in ablate:
                for a in range(A):
                    r = routes[a]
                    if r == "pe":
                        for c in range(CPS):
                            c0 = s0 + c * CH
                            sb_ps = psB.tile([128, CH], f32, tag="scB")
                            nc.tensor.matmul(sb_ps[:], lhsT=sel[:, a, :],
                                             rhs=scoreT[:, c * CH:(c + 1) * CH],
                                             start=True, stop=True)
                            nc.scalar.copy(
                                scb[:, a, c * CH:(c + 1) * CH], sb_ps[:])
                    elif r == "gp":
                        nc.gpsimd.partition_broadcast(
                            scb[:, a, :], scoreT[a:a + 1, :])
                    else:  # dma
                        nc.sync.dma_start(
                            out=scb[:, a, :],
                            in_=scoreT[a:a + 1, :]
                            .rearrange("p (x n) -> p x n", x=1)
                            .broadcast_to((1, 128, SC)))

                for a in range(A):
                    e = epool.tile([128, SPS, W], bf16, tag="e")
                    if "prod" not in ablate:
                        nc.vector.tensor_tensor(
                            out=e[:].rearrange("p k w -> p (k w)"),
                            in0=xpT[:, s0:s0 + SC],
                            in1=scb[:, a, :], op=OP.mult)
                    w = W
                    while w > 2:
                        w2 = w // 2
                        nc.vector.tensor_tensor(
                            out=e[:, :, :w2], in0=e[:, :, :w2],
                            in1=e[:, :, w2:w], op=OP.max)
                        w = w2
                    nc.vector.tensor_tensor(
                        out=segmax[:, a, s * SPS:(s + 1) * SPS],
                        in0=e[:, :, 0], in1=e[:, :, 1], op=OP.max)

          # --- per-segment aggregates -> proj [J, 64] ---
          with (
              tc.tile_pool(name="psC", bufs=2, space="PSUM") as psC,
              tc.tile_pool(name="small", bufs=2) as small,
          ):
              pm = psC.tile([J, 64], f32, tag="proj")
              for a in range(A):
                  nc.tensor.matmul(pm[:], lhsT=segsum[:, a, :], rhs=WmT[:, a, :],
                                   start=(a == 0), stop=(a == A - 1))
              px = psC.tile([J, 64], f32, tag="proj")
              for a in range(A):
                  nc.tensor.matmul(px[:], lhsT=segmax[:, a, :], rhs=WxxT[:, a, :],
                                   start=(a == 0), stop=(a == A - 1))
              proj = small.tile([J, 64], f32, tag="proj_sb")
              nc.vector.tensor_scalar(out=proj[:], in0=pm[:], scalar1=invc[:],
                                      scalar2=None, op0=OP.mult)
              nc.vector.tensor_tensor(out=proj[:], in0=proj[:], in1=px[:],
                                      op=OP.add)
              proj_r = small.tile([J, 64], bf16, tag="proj_r")
              nc.scalar.copy(proj_r[:], proj[:])

              # --- final projection + gather + relu ---
              with (
                  tc.tile_pool(name="psD", bufs=2, space="PSUM") as psD,
                  tc.tile_pool(name="gp", bufs=3) as gpp,
                  tc.tile_pool(name="yp", bufs=2) as yp,
              ):
                  if "phaseD" not in ablate:
                    for g in range(NCH):
                      c0 = g * CH
                      bw = W + fd[2 * g + 1]
                      gt = gpp.tile([J, CH], bf16, tag="gt")
                      dma_eng.dma_start(out=gt[:, :bw], in_=G_d[:, c0:c0 + bw])
                      yT_ps = psD.tile([64, CH], f32, tag="yT")
                      nc.tensor.matmul(yT_ps[:, :bw], lhsT=WxT[:],
                                       rhs=xT[:, c0:c0 + bw],
                                       start=True, stop=False)
                      nc.tensor.matmul(yT_ps[:, :bw], lhsT=WxpT[:],
                                       rhs=xpT[:, c0:c0 + bw],
                                       start=False, stop=False)
                      nc.tensor.matmul(yT_ps[:, :bw], lhsT=proj_r[:],
                                       rhs=gt[:, :bw],
                                       start=False, stop=True)
                      yT_sb = yp.tile([64, CH], bf16, tag="yT_sb")
                      nc.scalar.activation(yT_sb[:, :bw], yT_ps[:, :bw], AF.Relu)
                      nc.sync.dma_start(out=y_d[:, c0:c0 + bw], in_=yT_sb[:, :bw])
    nc.compile()
    return nc


def _prep(x, batch, W_in, b_in, W_a, b_a, W_out, b_out):
    x = np.asarray(x, np.float32)
    batch = np.asarray(batch).astype(np.int64)
    counts = np.bincount(batch, minlength=B).astype(np.int64)
    assert counts.max() <= W, f"segment count {counts.max()} > W={W}"
    seg_start = np.zeros(B + 1, np.int64)
    np.cumsum(counts, out=seg_start[1:])

    order = np.argsort(-counts, kind="stable")
    fd = np.zeros(J, np.int64)
    for j in range(J):
        mx = int(counts[order[j * NCORES:(j + 1) * NCORES]].max())
        fd[j] = min(W, max(32, mx))
    _prep.last_fd = tuple(int(v) for v in fd)

    W_out = np.asarray(W_out, np.float32)
    WmT = np.empty((128, A, 64), np.float32)
    WxxT = np.empty((128, A, 64), np.float32)
    for a in range(A):
        base = D_IN + F + a * 2 * F
        WmT[:, a, :] = W_out[:, base:base + F].T
        WxxT[:, a, :] = W_out[:, base + F:base + 2 * F].T

    for nm, v in (("b_in", b_in), ("b_a", b_a), ("b_out", b_out)):
        assert np.abs(np.asarray(v, np.float32)).max() == 0.0, f"{nm} != 0"

    shared = {
        "WinT": np.ascontiguousarray(np.asarray(W_in, np.float32).T).astype(BF16),
        "WaT": np.ascontiguousarray(np.asarray(W_a, np.float32).T).astype(BF16),
        "WxT": np.ascontiguousarray(W_out[:, :D_IN].T).astype(BF16),
        "WxpT": np.ascontiguousarray(W_out[:, D_IN:D_IN + F].T).astype(BF16),
        "WmT": np.ascontiguousarray(WmT.reshape(128, A * 64)),
        "WxxT": np.ascontiguousarray(WxxT.reshape(128, A * 64)),
        "sel": np.ascontiguousarray(
            np.repeat(np.eye(A, dtype=np.float32), 128, axis=1)).astype(BF16),
    }

    in_maps, gathers = [], []
    for c in range(NCORES):
        xT_c = np.zeros((128, NPAD), np.float32)
        G_c = np.zeros((J, NPAD), np.float32)
        invc_c = np.zeros((J, 1), np.float32)
        src_all, dst_all = [], []
        for j in range(J):
            seg = int(order[j * NCORES + c])
            n = int(counts[seg])
            invc_c[j] = 1.0 / max(n, 1)
            if n == 0:
                continue
            s0 = int(seg_start[seg])
            o = j * W
            src_all.append(np.arange(s0, s0 + n))
            dst_all.append(np.arange(o, o + n))
            G_c[j, o:o + n] = 1.0
        src = np.concatenate(src_all)
        dst = np.concatenate(dst_all)
        xT_c[:, dst] = x[src].T
        in_maps.append({"xT": xT_c.astype(BF16), "G": G_c.astype(BF16),
                        "invc": invc_c, **shared})
        gathers.append((src, dst))
    return None, NPAD, in_maps, gathers


def kernel(x, batch, num_segments, W_in, b_in, W_a, b_a, W_out, b_out,
           _trace=False):
    from concourse.bass_utils import run_bass_kernel_spmd

    assert int(num_segments) == B
    _, _, in_maps, gathers = _prep(
        x, batch, W_in, b_in, W_a, b_a, W_out, b_out)

    key = ("prog", _prep.last_fd)
    if key not in _cache:
        _cache[key] = _build_program(fd=_prep.last_fd)
    nc = _cache[key]

    res = run_bass_kernel_spmd(nc, in_maps, core_ids=list(range(NCORES)),
                               trace=_trace)
    out = np.empty((N_GLOBAL, D_OUT), np.float32)
    for c in range(NCORES):
        src, dst = gathers[c]
        y = np.asarray(res.results[c]["y"])  # [64, NPAD] bf16
        out[src] = y[:, dst].T.astype(np.float32)
    kernel._last_result = res
    return out
